# revision 1
# baseline (speedup 1.0000x reference)
"""CRF log-partition minus gold-path score, on 8 trn2 NeuronCores.

alpha (the log partition) is a product of 2M transition operators in
probability space. Mid-chain only tags {0,1,2} are reachable (no transitions
into START, none out of STOP, paths entering STOP die), so each step is a
3x3 operator A_t = E^T diag(e_t), E = exp(transitions[:3,:3] - mu), e_t =
exp(feats[t,:3]); the constant per-step scale mu (calibrated on 1k steps)
keeps bf16/f32 range without per-step renormalization.

Device kernel (SPMD, 8 cores x 250k steps): 20833 independent 12-step chunks
advance in lockstep, 42 chunks per 126x498 matmul round (block-diagonal
stationary of 42 E-copies loaded once, reused via ldweights=False); the
per-step diag(e_t) fuses into the PSUM->SBUF copy on VectorE; 3 interleaved
waves overlap PE/DVE; inputs arrive as one bf16 blob in 3 concurrent DMAs;
outputs stream per wave on two DMA engines. Host: exp(feats), f64 tree
combine of the 167k 3x3 chunk products, per-core 4-step remainder chunks,
exact START/STOP boundary handling, and the gold-path gather/sum.
Any device failure falls back to a pure-numpy tree reduction.
"""

import time

from contextlib import ExitStack

import numpy as np
import ml_dtypes

NTAGS = 5
START, STOP = 3, 4
NTE = 3

T_CORE = 250_000
L = 12
CBLK = 42
G = 498
NB = CBLK * G        # 20916
NFULL = T_CORE // L  # 20833 full chunks; one partial (PREM steps); rest padding
PREM = T_CORE - NFULL * L  # 4
W = 3
GW = G // W          # 166
P = CBLK * NTE       # 126
FD = GW * NTE        # 498
TS1 = 2
TS2 = 6
NWARM = 3
HDR = P + NTE        # blob header cols: ebd(126) + eye(3)


def _matmul_noload(pe, out, lhsT, rhs):
    from concourse import mybir
    ifmap_ap = pe.lower_ap(rhs.opt({0}), opt=False)
    weights_ap = pe.lower_ap(lhsT.opt({0}), opt=False, for_matmul_weights=True)
    out_ap = pe.lower_ap(out)
    return pe.add_instruction(mybir.InstMatmult(
        name=pe.bass.get_next_instruction_name(),
        replication_resolution=0, replication_shift_amnt=0,
        replication_num_rows=0,
        start_tensor_calc=True, stop_tensor_calc=True,
        ins=[ifmap_ap, weights_ap], outs=[out_ap],
        tile_position=(0, 0), tile_size=(128, 128),
        ldweights=False,
    ))


def _build_program():
    from concourse import bass, mybir
    global F32, BF16
    F32 = mybir.dt.float32
    BF16 = mybir.dt.bfloat16
    nc = bass.Bass(enable_partition_id=False)
    blob_in = nc.declare_dram_parameter("blob", [P, HDR + L * G], BF16, isOutput=False)
    out_last = nc.declare_dram_parameter("out_last", [P, G * NTE], BF16, isOutput=True)

    ctx = ExitStack()
    with ctx:
        blob_sb = ctx.enter_context(nc.sbuf_tensor("blob_sb", [P, HDR + L * G], BF16))
        rhs = [
            [
                ctx.enter_context(nc.sbuf_tensor(f"rhs_{w}_{p}", [P, GW, NTE], BF16))
                for p in range(2)
            ]
            for w in range(W)
        ]
        ps = [
            [
                ctx.enter_context(nc.psum_tensor(f"ps_{w}_{p}", [P, FD], F32))
                for p in range(2)
            ]
            for w in range(W)
        ]
        ps_warm = ctx.enter_context(nc.psum_tensor("ps_warm", [P, FD], F32))
        dma_in_sem = ctx.enter_context(nc.semaphore("dma_in_sem"))
        s_in1 = ctx.enter_context(nc.semaphore("s_in1"))
        s_in2 = ctx.enter_context(nc.semaphore("s_in2"))
        s_in3 = ctx.enter_context(nc.semaphore("s_in3"))
        rhs_sem = ctx.enter_context(nc.semaphore("rhs_sem"))
        mm_sem = ctx.enter_context(nc.semaphore("mm_sem"))
        block = ctx.enter_context(nc.Block())

        ebd_sb = blob_sb[:, :P]
        eye_sb = blob_sb[:, P : P + NTE]

        def e_bc(t, w):
            base = HDR + t * G + w * GW
            return blob_sb[:, base : base + GW].unsqueeze(2).broadcast_to(
                [P, GW, NTE]
            )

        SP1 = HDR + TS1 * G
        SP2 = HDR + TS2 * G

        @block.sync
        def _(eng):
            eng.dma_start(out=blob_sb[:, :SP1], in_=blob_in[:, :SP1]).then_inc(
                s_in1, 16
            )
            eng.dma_start(out=blob_sb[:, SP2:], in_=blob_in[:, SP2:]).then_inc(
                s_in3, 16
            )
            eng.wait_ge(rhs_sem, (L - 1) * W + 2)
            eng.dma_start(
                out=out_last[:, FD : 2 * FD],
                in_=rhs[1][(L - 1) % 2][:].rearrange("p g j -> p (g j)"),
            ).then_inc(dma_in_sem, 16)

        @block.vector
        def _(eng):
            eng.wait_ge(s_in1, 16)
            for w in range(W):
                eng.tensor_tensor(
                    rhs[w][0][:],
                    e_bc(0, w),
                    eye_sb.unsqueeze(1).broadcast_to([P, GW, NTE]),
                    mybir.AluOpType.mult,
                ).then_inc(rhs_sem, 1)
            for t in range(1, L):
                if t == TS1:
                    eng.wait_ge(s_in2, 16)
                if t == TS2:
                    eng.wait_ge(s_in3, 16)
                for w in range(W):
                    eng.wait_ge(mm_sem, (t - 1) * W + w + 1)
                    eng.tensor_tensor(
                        rhs[w][t % 2][:],
                        ps[w][(t - 1) % 2][:].rearrange("p (g j) -> p g j", j=NTE),
                        e_bc(t, w),
                        mybir.AluOpType.mult,
                    ).then_inc(rhs_sem, 1)

        @block.tensor
        def _(eng):
            eng.wait_ge(s_in1, 16)
            eng.matmul(ps_warm[:, :P], ebd_sb, ebd_sb, start=True, stop=True)
            for _ in range(NWARM - 1):
                _matmul_noload(eng, ps_warm[:, :P], ebd_sb, ebd_sb)
            for t in range(L - 1):
                for w in range(W):
                    eng.wait_ge(rhs_sem, t * W + w + 1)
                    _matmul_noload(
                        eng,
                        ps[w][t % 2][:],
                        ebd_sb,
                        rhs[w][t % 2][:].rearrange("p g j -> p (g j)"),
                    ).then_inc(mm_sem, 1)

        @block.gpsimd
        def _(eng):
            eng.dma_start(
                out=blob_sb[:, SP1:SP2], in_=blob_in[:, SP1:SP2]
            ).then_inc(s_in2, 16)
            for w in (0, 2):
                eng.wait_ge(rhs_sem, (L - 1) * W + w + 1)
                eng.dma_start(
                    out=out_last[:, w * FD : (w + 1) * FD],
                    in_=rhs[w][(L - 1) % 2][:].rearrange("p g j -> p (g j)"),
                ).then_inc(dma_in_sem, 16)
            eng.wait_ge(s_in2, 16)
            eng.wait_ge(dma_in_sem, 16 * W)

    return nc


def _bf16(x):
    return np.asarray(x, dtype=ml_dtypes.bfloat16)


def _chunk_lens():
    return [L] * NFULL + [PREM]


def _prep_inputs(feats, transitions, mu):
    E3 = np.exp(transitions[:NTE, :NTE].astype(np.float64) - mu).astype(np.float32)
    hdr = np.zeros((P, HDR), np.float32)
    for c in range(CBLK):
        hdr[NTE * c : NTE * (c + 1), NTE * c : NTE * (c + 1)] = E3
        hdr[NTE * c : NTE * (c + 1), P : P + NTE] = np.eye(NTE, dtype=np.float32)
    hdr16 = _bf16(hdr)

    f = np.asarray(feats, np.float32).reshape(8, T_CORE, NTAGS)
    in_maps = []
    for k in range(8):
        e3 = np.exp(f[k, :, :NTE].astype(np.float32))
        e_full = np.ones((NB, L, NTE), np.float32)
        e_full[:NFULL] = e3[: NFULL * L].reshape(NFULL, L, NTE)
        e_full[NFULL, :PREM] = e3[NFULL * L :]
        er = np.ascontiguousarray(
            e_full.reshape(CBLK, G, L, NTE).transpose(0, 3, 2, 1)
        ).reshape(P, L * G)
        blob = np.concatenate([hdr16, _bf16(er)], axis=1)
        in_maps.append({"blob": blob})
    return in_maps


def _calibrate_mu(feats, transitions, n=1000):
    E = np.exp(transitions.astype(np.float64))
    v = np.ones(NTAGS)
    acc = 0.0
    for t in range(n):
        v = (E.T * np.exp(feats[t].astype(np.float64))) @ v
        m = v.max()
        v /= m
        acc += np.log(m)
    return acc / n


def _extract_chunks(res_last):
    u = np.asarray(res_last, np.float64).reshape(CBLK, NTE, W, GW, NTE)
    return u.transpose(0, 2, 3, 1, 4).reshape(NB, NTE, NTE)[:NFULL]


def _combine_host(mats_per_core, feats, transitions, mu):
    """mats_per_core: list of 8 arrays [NFULL,3,3] (device full chunks).
    The per-core partial chunk (PREM steps) is computed here in f64; the
    global last chunk becomes the row-START boundary vector."""
    T = feats.shape[0]
    tr = np.asarray(transitions, np.float64)
    E3 = np.exp(tr[:NTE, :NTE] - mu)

    def host_chunk(off, ln):
        fb = np.exp(feats[off : off + ln, :NTE].astype(np.float64))
        U = np.eye(NTE)
        for t in range(ln):
            U = E3.T @ (fb[t][:, None] * U)
        return U

    per_core = []
    for k in range(8):
        # device ships rhs_{L-1}; apply the final E^T here (exact, f64)
        per_core.append(np.einsum("ki,bkj->bij", E3, mats_per_core[k]))
        off = k * T_CORE + NFULL * L
        if k < 7:
            per_core.append(host_chunk(off, PREM)[None])
    mats = np.concatenate(per_core, 0)

    # global last chunk: z^T = rowSTART(A_{T-1}) * prod(A_t desc) over PREM steps
    off_last = T - PREM
    e3 = np.exp(feats[off_last:, :NTE].astype(np.float64))
    z = np.exp(tr[:NTE, START] - mu) * e3[-1]
    for t in range(PREM - 2, -1, -1):
        z = e3[t] * (E3 @ z)
    Z = np.zeros((NTE, NTE))
    Z[0] = z

    Pm = np.concatenate([mats, Z[None]], axis=0)
    bad = ~np.isfinite(Pm).all(axis=(1, 2)) | (np.abs(Pm).max(axis=(1, 2)) <= 0)
    if bad.any():
        per_core_n = NFULL + 1
        for m in np.nonzero(bad)[0]:
            if m == len(Pm) - 1:
                raise RuntimeError("host z chunk non-finite")
            k, local = divmod(int(m), per_core_n)
            off = k * T_CORE + L * local
            ln = L if local < NFULL else PREM
            Pm[m] = host_chunk(off, ln)
    ls = 0.0
    n = np.abs(Pm).max(axis=(1, 2))
    Pm = Pm / n[:, None, None]
    ls += np.log(n).sum()
    while Pm.shape[0] > 1:
        nn_ = Pm.shape[0]
        m = nn_ - (nn_ % 2)
        C = np.einsum("bik,bkj->bij", Pm[1:m:2], Pm[0:m:2])
        if nn_ % 2:
            C = np.concatenate([C, Pm[m:]], 0)
        sc = np.abs(C).max(axis=(1, 2))
        sc = np.where(sc > 0, sc, 1.0)
        C /= sc[:, None, None]
        ls += np.log(sc).sum()
        Pm = C
    B = Pm[0]
    wt = np.exp(tr[STOP, :NTE])
    val = float(B[0] @ wt)
    return float(np.log(val) + ls + T * mu)


def _gold_score(feats, tags, transitions):
    T = feats.shape[0]
    tags_i = np.asarray(tags).astype(np.int64)
    tr = np.asarray(transitions, np.float64)
    prev = np.concatenate([np.array([START], np.int64), tags_i[:-1]])
    trans_score = tr[tags_i, prev].sum()
    emit_score = np.asarray(feats, np.float64)[np.arange(T), tags_i].sum()
    return trans_score + emit_score + tr[STOP, tags_i[-1]]


_PROGRAM = None


def _alpha_device(feats, transitions):
    global _PROGRAM
    from concourse.bass_utils import run_bass_kernel_spmd

    if _PROGRAM is None:
        _PROGRAM = _build_program()
    mu = _calibrate_mu(feats, transitions)
    in_maps = _prep_inputs(feats, transitions, mu)
    res = run_bass_kernel_spmd(_PROGRAM, in_maps, list(range(8)))
    mats = [_extract_chunks(res.results[k]["out_last"]) for k in range(8)]
    return _combine_host(mats, feats, transitions, mu)


def _alpha_numpy(feats, transitions):
    """Fallback: pairwise tree reduction over 5x5 operators (pure numpy)."""
    trans = transitions.astype(np.float32)
    M = trans[None, :, :] + feats[:, :, None].astype(np.float32)
    s = M.max(axis=(1, 2))
    Pr = np.exp(M - s[:, None, None])
    ls = s.astype(np.float64).sum()
    while Pr.shape[0] > 1:
        n = Pr.shape[0]
        m = n - (n % 2)
        C = (Pr[1:m:2][:, :, :, None] * Pr[0:m:2][:, None, :, :]).sum(axis=2)
        if n % 2:
            C = np.concatenate([C, Pr[m:]], axis=0)
        sc = C.max(axis=(1, 2))
        sc = np.where(sc > 0, sc, 1.0).astype(np.float32)
        C /= sc[:, None, None]
        ls += np.log(sc.astype(np.float64)).sum()
        Pr = C
    u = Pr[0][:, START].astype(np.float64)
    w = np.exp(trans[STOP].astype(np.float64))
    return float(np.log((w * u).sum()) + ls)


def kernel(feats, tags, transitions):
    feats = np.asarray(feats, np.float32)
    tags = np.asarray(tags)
    transitions = np.asarray(transitions, np.float32)

    alpha = None
    if feats.shape[0] == 2_000_000:
        for attempt in range(2):
            try:
                alpha = _alpha_device(feats, transitions)
                break
            except Exception:
                alpha = None
                if attempt == 0:
                    time.sleep(20)  # transient device wedges clear on retry
    if alpha is None:
        alpha = _alpha_numpy(feats, transitions)

    gold = _gold_score(feats, tags, transitions)
    return np.asarray(alpha - gold, dtype=np.float32)



# revision 2
# speedup vs baseline: 1.0770x; 1.0770x over previous
"""CRF log-partition minus gold-path score, on 8 trn2 NeuronCores.

alpha (the log partition) is a product of 2M transition operators in
probability space. Mid-chain only tags {0,1,2} are reachable (no transitions
into START, none out of STOP, paths entering STOP die), so each step is a
3x3 operator A_t = E^T diag(e_t), E = exp(transitions[:3,:3] - mu), e_t =
exp(feats[t,:3]); the constant per-step scale mu (calibrated on 1k steps)
keeps bf16/f32 range without per-step renormalization.

Device kernel (SPMD, 8 cores x 250k steps): 20833 independent 12-step chunks
advance in lockstep, 42 chunks per 126x498 matmul round (block-diagonal
stationary of 42 E-copies loaded once, reused via ldweights=False); the
per-step diag(e_t) fuses into the PSUM->SBUF copy on VectorE; 3 interleaved
waves overlap PE/DVE; inputs arrive as one bf16 blob in 3 concurrent DMAs;
outputs stream per wave on two DMA engines. Host: exp(feats), f64 tree
combine of the 167k 3x3 chunk products, per-core 4-step remainder chunks,
exact START/STOP boundary handling, and the gold-path gather/sum.
Any device failure falls back to a pure-numpy tree reduction.
"""

import time

from contextlib import ExitStack

import numpy as np
import ml_dtypes

NTAGS = 5
START, STOP = 3, 4
NTE = 3

T_CORE = 250_000
L = 12
CBLK = 42
G = 498
NB = CBLK * G        # 20916
NFULL = T_CORE // L  # 20833 full chunks; one partial (PREM steps); rest padding
PREM = T_CORE - NFULL * L  # 4
W = 3
GW = G // W          # 166
P = CBLK * NTE       # 126
FD = GW * NTE        # 498
TS1 = 2
TS2 = 6
NWARM = 3
HDR = P + NTE        # blob header cols: ebd(126) + eye(3)


def _matmul_noload(pe, out, lhsT, rhs):
    from concourse import mybir
    ifmap_ap = pe.lower_ap(rhs.opt({0}), opt=False)
    weights_ap = pe.lower_ap(lhsT.opt({0}), opt=False, for_matmul_weights=True)
    out_ap = pe.lower_ap(out)
    return pe.add_instruction(mybir.InstMatmult(
        name=pe.bass.get_next_instruction_name(),
        replication_resolution=0, replication_shift_amnt=0,
        replication_num_rows=0,
        start_tensor_calc=True, stop_tensor_calc=True,
        ins=[ifmap_ap, weights_ap], outs=[out_ap],
        tile_position=(0, 0), tile_size=(128, 128),
        ldweights=False,
    ))


def _build_program():
    from concourse import bass, mybir
    global F32, BF16
    F32 = mybir.dt.float32
    BF16 = mybir.dt.bfloat16
    nc = bass.Bass(enable_partition_id=False)
    blob_in = nc.declare_dram_parameter("blob", [P, HDR + L * G], BF16, isOutput=False)
    out_last = nc.declare_dram_parameter("out_last", [P, G * NTE], BF16, isOutput=True)

    ctx = ExitStack()
    with ctx:
        blob_sb = ctx.enter_context(nc.sbuf_tensor("blob_sb", [P, HDR + L * G], BF16))
        rhs = [
            [
                ctx.enter_context(nc.sbuf_tensor(f"rhs_{w}_{p}", [P, GW, NTE], BF16))
                for p in range(2)
            ]
            for w in range(W)
        ]
        ps = [
            [
                ctx.enter_context(nc.psum_tensor(f"ps_{w}_{p}", [P, FD], F32))
                for p in range(2)
            ]
            for w in range(W)
        ]
        ps_warm = ctx.enter_context(nc.psum_tensor("ps_warm", [P, FD], F32))
        dma_in_sem = ctx.enter_context(nc.semaphore("dma_in_sem"))
        s_in1 = ctx.enter_context(nc.semaphore("s_in1"))
        s_in2 = ctx.enter_context(nc.semaphore("s_in2"))
        s_in3 = ctx.enter_context(nc.semaphore("s_in3"))
        rhs_sem = ctx.enter_context(nc.semaphore("rhs_sem"))
        mm_sem = ctx.enter_context(nc.semaphore("mm_sem"))
        block = ctx.enter_context(nc.Block())

        ebd_sb = blob_sb[:, :P]
        eye_sb = blob_sb[:, P : P + NTE]

        def e_bc(t, w):
            base = HDR + t * G + w * GW
            return blob_sb[:, base : base + GW].unsqueeze(2).broadcast_to(
                [P, GW, NTE]
            )

        SP1 = HDR + TS1 * G
        SP2 = HDR + TS2 * G

        @block.sync
        def _(eng):
            eng.dma_start(
                out=blob_sb[:, : SP1 // 2], in_=blob_in[:, : SP1 // 2]
            ).then_inc(s_in1, 16)
            eng.wait_ge(rhs_sem, (L - 1) * W + 2)
            eng.dma_start(
                out=out_last[:, FD : 2 * FD],
                in_=rhs[1][(L - 1) % 2][:].rearrange("p g j -> p (g j)"),
            ).then_inc(dma_in_sem, 16)

        @block.scalar
        def _(eng):
            eng.dma_start(
                out=blob_sb[:, SP1 // 2 : SP1], in_=blob_in[:, SP1 // 2 : SP1]
            ).then_inc(s_in1, 16)
            eng.dma_start(out=blob_sb[:, SP2:], in_=blob_in[:, SP2:]).then_inc(
                s_in3, 16
            )

        @block.vector
        def _(eng):
            eng.wait_ge(s_in1, 32)
            for w in range(W):
                eng.tensor_tensor(
                    rhs[w][0][:],
                    e_bc(0, w),
                    eye_sb.unsqueeze(1).broadcast_to([P, GW, NTE]),
                    mybir.AluOpType.mult,
                ).then_inc(rhs_sem, 1)
            for t in range(1, L):
                if t == TS1:
                    eng.wait_ge(s_in2, 16)
                if t == TS2:
                    eng.wait_ge(s_in3, 16)
                for w in range(W):
                    eng.wait_ge(mm_sem, (t - 1) * W + w + 1)
                    eng.tensor_tensor(
                        rhs[w][t % 2][:],
                        ps[w][(t - 1) % 2][:].rearrange("p (g j) -> p g j", j=NTE),
                        e_bc(t, w),
                        mybir.AluOpType.mult,
                    ).then_inc(rhs_sem, 1)

        @block.tensor
        def _(eng):
            eng.wait_ge(s_in1, 16)
            eng.matmul(ps_warm[:, :P], ebd_sb, ebd_sb, start=True, stop=True)
            for _ in range(NWARM - 1):
                _matmul_noload(eng, ps_warm[:, :P], ebd_sb, ebd_sb)
            for t in range(L - 1):
                for w in range(W):
                    eng.wait_ge(rhs_sem, t * W + w + 1)
                    _matmul_noload(
                        eng,
                        ps[w][t % 2][:],
                        ebd_sb,
                        rhs[w][t % 2][:].rearrange("p g j -> p (g j)"),
                    ).then_inc(mm_sem, 1)

        @block.gpsimd
        def _(eng):
            eng.dma_start(
                out=blob_sb[:, SP1:SP2], in_=blob_in[:, SP1:SP2]
            ).then_inc(s_in2, 16)
            for w in (0, 2):
                eng.wait_ge(rhs_sem, (L - 1) * W + w + 1)
                eng.dma_start(
                    out=out_last[:, w * FD : (w + 1) * FD],
                    in_=rhs[w][(L - 1) % 2][:].rearrange("p g j -> p (g j)"),
                ).then_inc(dma_in_sem, 16)
            eng.wait_ge(s_in2, 16)
            eng.wait_ge(dma_in_sem, 16 * W)

    return nc


def _bf16(x):
    return np.asarray(x, dtype=ml_dtypes.bfloat16)


def _chunk_lens():
    return [L] * NFULL + [PREM]


def _prep_inputs(feats, transitions, mu):
    E3 = np.exp(transitions[:NTE, :NTE].astype(np.float64) - mu).astype(np.float32)
    hdr = np.zeros((P, HDR), np.float32)
    for c in range(CBLK):
        hdr[NTE * c : NTE * (c + 1), NTE * c : NTE * (c + 1)] = E3
        hdr[NTE * c : NTE * (c + 1), P : P + NTE] = np.eye(NTE, dtype=np.float32)
    hdr16 = _bf16(hdr)

    f = np.asarray(feats, np.float32).reshape(8, T_CORE, NTAGS)
    in_maps = []
    for k in range(8):
        e3 = np.exp(f[k, :, :NTE].astype(np.float32))
        e_full = np.ones((NB, L, NTE), np.float32)
        e_full[:NFULL] = e3[: NFULL * L].reshape(NFULL, L, NTE)
        e_full[NFULL, :PREM] = e3[NFULL * L :]
        er = np.ascontiguousarray(
            e_full.reshape(CBLK, G, L, NTE).transpose(0, 3, 2, 1)
        ).reshape(P, L * G)
        blob = np.concatenate([hdr16, _bf16(er)], axis=1)
        in_maps.append({"blob": blob})
    return in_maps


def _calibrate_mu(feats, transitions, n=1000):
    E = np.exp(transitions.astype(np.float64))
    v = np.ones(NTAGS)
    acc = 0.0
    for t in range(n):
        v = (E.T * np.exp(feats[t].astype(np.float64))) @ v
        m = v.max()
        v /= m
        acc += np.log(m)
    return acc / n


def _extract_chunks(res_last):
    u = np.asarray(res_last, np.float64).reshape(CBLK, NTE, W, GW, NTE)
    return u.transpose(0, 2, 3, 1, 4).reshape(NB, NTE, NTE)[:NFULL]


def _combine_host(mats_per_core, feats, transitions, mu):
    """mats_per_core: list of 8 arrays [NFULL,3,3] (device full chunks).
    The per-core partial chunk (PREM steps) is computed here in f64; the
    global last chunk becomes the row-START boundary vector."""
    T = feats.shape[0]
    tr = np.asarray(transitions, np.float64)
    E3 = np.exp(tr[:NTE, :NTE] - mu)

    def host_chunk(off, ln):
        fb = np.exp(feats[off : off + ln, :NTE].astype(np.float64))
        U = np.eye(NTE)
        for t in range(ln):
            U = E3.T @ (fb[t][:, None] * U)
        return U

    per_core = []
    for k in range(8):
        # device ships rhs_{L-1}; apply the final E^T here (exact, f64)
        per_core.append(np.einsum("ki,bkj->bij", E3, mats_per_core[k]))
        off = k * T_CORE + NFULL * L
        if k < 7:
            per_core.append(host_chunk(off, PREM)[None])
    mats = np.concatenate(per_core, 0)

    # global last chunk: z^T = rowSTART(A_{T-1}) * prod(A_t desc) over PREM steps
    off_last = T - PREM
    e3 = np.exp(feats[off_last:, :NTE].astype(np.float64))
    z = np.exp(tr[:NTE, START] - mu) * e3[-1]
    for t in range(PREM - 2, -1, -1):
        z = e3[t] * (E3 @ z)
    Z = np.zeros((NTE, NTE))
    Z[0] = z

    Pm = np.concatenate([mats, Z[None]], axis=0)
    bad = ~np.isfinite(Pm).all(axis=(1, 2)) | (np.abs(Pm).max(axis=(1, 2)) <= 0)
    if bad.any():
        per_core_n = NFULL + 1
        for m in np.nonzero(bad)[0]:
            if m == len(Pm) - 1:
                raise RuntimeError("host z chunk non-finite")
            k, local = divmod(int(m), per_core_n)
            off = k * T_CORE + L * local
            ln = L if local < NFULL else PREM
            Pm[m] = host_chunk(off, ln)
    ls = 0.0
    n = np.abs(Pm).max(axis=(1, 2))
    Pm = Pm / n[:, None, None]
    ls += np.log(n).sum()
    while Pm.shape[0] > 1:
        nn_ = Pm.shape[0]
        m = nn_ - (nn_ % 2)
        C = np.einsum("bik,bkj->bij", Pm[1:m:2], Pm[0:m:2])
        if nn_ % 2:
            C = np.concatenate([C, Pm[m:]], 0)
        sc = np.abs(C).max(axis=(1, 2))
        sc = np.where(sc > 0, sc, 1.0)
        C /= sc[:, None, None]
        ls += np.log(sc).sum()
        Pm = C
    B = Pm[0]
    wt = np.exp(tr[STOP, :NTE])
    val = float(B[0] @ wt)
    return float(np.log(val) + ls + T * mu)


def _gold_score(feats, tags, transitions):
    T = feats.shape[0]
    tags_i = np.asarray(tags).astype(np.int64)
    tr = np.asarray(transitions, np.float64)
    prev = np.concatenate([np.array([START], np.int64), tags_i[:-1]])
    trans_score = tr[tags_i, prev].sum()
    emit_score = np.asarray(feats, np.float64)[np.arange(T), tags_i].sum()
    return trans_score + emit_score + tr[STOP, tags_i[-1]]


_PROGRAM = None


def _alpha_device(feats, transitions):
    global _PROGRAM
    from concourse.bass_utils import run_bass_kernel_spmd

    if _PROGRAM is None:
        _PROGRAM = _build_program()
    mu = _calibrate_mu(feats, transitions)
    in_maps = _prep_inputs(feats, transitions, mu)
    res = run_bass_kernel_spmd(_PROGRAM, in_maps, list(range(8)))
    mats = [_extract_chunks(res.results[k]["out_last"]) for k in range(8)]
    return _combine_host(mats, feats, transitions, mu)


def _alpha_numpy(feats, transitions):
    """Fallback: pairwise tree reduction over 5x5 operators (pure numpy)."""
    trans = transitions.astype(np.float32)
    M = trans[None, :, :] + feats[:, :, None].astype(np.float32)
    s = M.max(axis=(1, 2))
    Pr = np.exp(M - s[:, None, None])
    ls = s.astype(np.float64).sum()
    while Pr.shape[0] > 1:
        n = Pr.shape[0]
        m = n - (n % 2)
        C = (Pr[1:m:2][:, :, :, None] * Pr[0:m:2][:, None, :, :]).sum(axis=2)
        if n % 2:
            C = np.concatenate([C, Pr[m:]], axis=0)
        sc = C.max(axis=(1, 2))
        sc = np.where(sc > 0, sc, 1.0).astype(np.float32)
        C /= sc[:, None, None]
        ls += np.log(sc.astype(np.float64)).sum()
        Pr = C
    u = Pr[0][:, START].astype(np.float64)
    w = np.exp(trans[STOP].astype(np.float64))
    return float(np.log((w * u).sum()) + ls)


def kernel(feats, tags, transitions):
    feats = np.asarray(feats, np.float32)
    tags = np.asarray(tags)
    transitions = np.asarray(transitions, np.float32)

    alpha = None
    if feats.shape[0] == 2_000_000:
        for attempt in range(2):
            try:
                alpha = _alpha_device(feats, transitions)
                break
            except Exception:
                alpha = None
                if attempt == 0:
                    time.sleep(20)  # transient device wedges clear on retry
    if alpha is None:
        alpha = _alpha_numpy(feats, transitions)

    gold = _gold_score(feats, tags, transitions)
    return np.asarray(alpha - gold, dtype=np.float32)



# revision 4
# speedup vs baseline: 1.1331x; 1.0521x over previous
"""CRF log-partition minus gold-path score, on 8 trn2 NeuronCores.

alpha (the log partition) is a product of 2M transition operators in
probability space. Mid-chain only tags {0,1,2} are reachable (no transitions
into START, none out of STOP, paths entering STOP die), so each step is a
3x3 operator A_t = E^T diag(e_t), E = exp(transitions[:3,:3] - mu), e_t =
exp(feats[t,:3]); the constant per-step scale mu (calibrated on 1k steps)
keeps bf16/f32 range without per-step renormalization.

Device kernel (SPMD, 8 cores x 250k steps): 20833 independent 12-step chunks
advance in lockstep, 42 chunks per 126x498 matmul round (block-diagonal
stationary of 42 E-copies loaded once, reused via ldweights=False); the
per-step diag(e_t) fuses into the PSUM->SBUF copy on VectorE; 3 interleaved
waves overlap PE/DVE; inputs arrive as one bf16 blob in 3 concurrent DMAs;
outputs stream per wave on two DMA engines. Host: exp(feats), f64 tree
combine of the 167k 3x3 chunk products, per-core 4-step remainder chunks,
exact START/STOP boundary handling, and the gold-path gather/sum.
Any device failure falls back to a pure-numpy tree reduction.
"""

import time

from contextlib import ExitStack

import numpy as np
import ml_dtypes

NTAGS = 5
START, STOP = 3, 4
NTE = 3

T_CORE = 250_000
L = 12
CBLK = 42
G = 498
NB = CBLK * G        # 20916
NFULL = T_CORE // L  # 20833 full chunks; one partial (PREM steps); rest padding
PREM = T_CORE - NFULL * L  # 4
W = 3
GW = G // W          # 166
P = CBLK * NTE       # 126
FD = GW * NTE        # 498
TS1 = 2
TS2 = 6
NWARM = 3
HDR = P + NTE        # blob header cols: ebd(126) + eye(3)


def _matmul_noload(pe, out, lhsT, rhs):
    from concourse import mybir
    ifmap_ap = pe.lower_ap(rhs.opt({0}), opt=False)
    weights_ap = pe.lower_ap(lhsT.opt({0}), opt=False, for_matmul_weights=True)
    out_ap = pe.lower_ap(out)
    return pe.add_instruction(mybir.InstMatmult(
        name=pe.bass.get_next_instruction_name(),
        replication_resolution=0, replication_shift_amnt=0,
        replication_num_rows=0,
        start_tensor_calc=True, stop_tensor_calc=True,
        ins=[ifmap_ap, weights_ap], outs=[out_ap],
        tile_position=(0, 0), tile_size=(128, 128),
        ldweights=False,
    ))


def _build_program():
    from concourse import bass, mybir
    global F32, BF16
    F32 = mybir.dt.float32
    BF16 = mybir.dt.bfloat16
    nc = bass.Bass(enable_partition_id=False)
    blob_in = nc.declare_dram_parameter("blob", [P, HDR + L * G], BF16, isOutput=False)
    out_last = nc.declare_dram_parameter("out_last", [P, G * NTE], BF16, isOutput=True)

    ctx = ExitStack()
    with ctx:
        blob_sb = ctx.enter_context(nc.sbuf_tensor("blob_sb", [P, HDR + L * G], BF16))
        rhs = [
            [
                ctx.enter_context(nc.sbuf_tensor(f"rhs_{w}_{p}", [P, GW, NTE], BF16))
                for p in range(2)
            ]
            for w in range(W)
        ]
        ps = [
            [
                ctx.enter_context(nc.psum_tensor(f"ps_{w}_{p}", [P, FD], F32))
                for p in range(2)
            ]
            for w in range(W)
        ]
        ps_warm = ctx.enter_context(nc.psum_tensor("ps_warm", [P, FD], F32))
        dma_in_sem = ctx.enter_context(nc.semaphore("dma_in_sem"))
        s_in1 = ctx.enter_context(nc.semaphore("s_in1"))
        s_in1b = ctx.enter_context(nc.semaphore("s_in1b"))
        s_in2 = ctx.enter_context(nc.semaphore("s_in2"))
        s_in3 = ctx.enter_context(nc.semaphore("s_in3"))
        rhs_sem = ctx.enter_context(nc.semaphore("rhs_sem"))
        mm_sem = ctx.enter_context(nc.semaphore("mm_sem"))
        block = ctx.enter_context(nc.Block())

        ebd_sb = blob_sb[:, :P]
        eye_sb = blob_sb[:, P : P + NTE]

        def e_bc(t, w):
            base = HDR + t * G + w * GW
            return blob_sb[:, base : base + GW].unsqueeze(2).broadcast_to(
                [P, GW, NTE]
            )

        SP1 = HDR + TS1 * G
        SP2 = HDR + TS2 * G

        SPA = HDR + G
        @block.sync
        def _(eng):
            eng.dma_start(out=blob_sb[:, :SPA], in_=blob_in[:, :SPA]).then_inc(
                s_in1, 16
            )
            eng.dma_start(
                out=blob_sb[:, SPA:SP1], in_=blob_in[:, SPA:SP1]
            ).then_inc(s_in1b, 16)
            eng.dma_start(out=blob_sb[:, SP2:], in_=blob_in[:, SP2:]).then_inc(
                s_in3, 16
            )
            eng.wait_ge(rhs_sem, (L - 1) * W + 2)
            eng.dma_start(
                out=out_last[:, FD : 2 * FD],
                in_=rhs[1][(L - 1) % 2][:].rearrange("p g j -> p (g j)"),
            ).then_inc(dma_in_sem, 16)

        @block.vector
        def _(eng):
            eng.wait_ge(s_in1, 16)
            for w in range(W):
                eng.tensor_tensor(
                    rhs[w][0][:],
                    e_bc(0, w),
                    eye_sb.unsqueeze(1).broadcast_to([P, GW, NTE]),
                    mybir.AluOpType.mult,
                ).then_inc(rhs_sem, 1)
            for t in range(1, L):
                if t == 1:
                    eng.wait_ge(s_in1b, 16)
                if t == TS1:
                    eng.wait_ge(s_in2, 16)
                if t == TS2:
                    eng.wait_ge(s_in3, 16)
                for w in range(W):
                    eng.wait_ge(mm_sem, (t - 1) * W + w + 1)
                    eng.tensor_tensor(
                        rhs[w][t % 2][:],
                        ps[w][(t - 1) % 2][:].rearrange("p (g j) -> p g j", j=NTE),
                        e_bc(t, w),
                        mybir.AluOpType.mult,
                    ).then_inc(rhs_sem, 1)

        @block.tensor
        def _(eng):
            eng.wait_ge(s_in1, 16)
            eng.matmul(ps_warm[:, :P], ebd_sb, ebd_sb, start=True, stop=True)
            for _ in range(NWARM - 1):
                _matmul_noload(eng, ps_warm[:, :P], ebd_sb, ebd_sb)
            for t in range(L - 1):
                for w in range(W):
                    eng.wait_ge(rhs_sem, t * W + w + 1)
                    _matmul_noload(
                        eng,
                        ps[w][t % 2][:],
                        ebd_sb,
                        rhs[w][t % 2][:].rearrange("p g j -> p (g j)"),
                    ).then_inc(mm_sem, 1)

        @block.scalar
        def _(eng):
            eng.wait_ge(rhs_sem, (L - 1) * W + 3)
            eng.dma_start(
                out=out_last[:, 2 * FD : 3 * FD],
                in_=rhs[2][(L - 1) % 2][:].rearrange("p g j -> p (g j)"),
            ).then_inc(dma_in_sem, 16)

        @block.gpsimd
        def _(eng):
            eng.dma_start(
                out=blob_sb[:, SP1:SP2], in_=blob_in[:, SP1:SP2]
            ).then_inc(s_in2, 16)
            eng.wait_ge(rhs_sem, (L - 1) * W + 1)
            eng.dma_start(
                out=out_last[:, 0:FD],
                in_=rhs[0][(L - 1) % 2][:].rearrange("p g j -> p (g j)"),
            ).then_inc(dma_in_sem, 16)
            eng.wait_ge(s_in2, 16)
            eng.wait_ge(dma_in_sem, 16 * W)

    return nc


def _bf16(x):
    return np.asarray(x, dtype=ml_dtypes.bfloat16)


def _chunk_lens():
    return [L] * NFULL + [PREM]


def _prep_inputs(feats, transitions, mu):
    E3 = np.exp(transitions[:NTE, :NTE].astype(np.float64) - mu).astype(np.float32)
    hdr = np.zeros((P, HDR), np.float32)
    for c in range(CBLK):
        hdr[NTE * c : NTE * (c + 1), NTE * c : NTE * (c + 1)] = E3
        hdr[NTE * c : NTE * (c + 1), P : P + NTE] = np.eye(NTE, dtype=np.float32)
    hdr16 = _bf16(hdr)

    f = np.asarray(feats, np.float32).reshape(8, T_CORE, NTAGS)
    in_maps = []
    for k in range(8):
        e3 = np.exp(f[k, :, :NTE].astype(np.float32))
        e_full = np.ones((NB, L, NTE), np.float32)
        e_full[:NFULL] = e3[: NFULL * L].reshape(NFULL, L, NTE)
        e_full[NFULL, :PREM] = e3[NFULL * L :]
        er = np.ascontiguousarray(
            e_full.reshape(CBLK, G, L, NTE).transpose(0, 3, 2, 1)
        ).reshape(P, L * G)
        blob = np.concatenate([hdr16, _bf16(er)], axis=1)
        in_maps.append({"blob": blob})
    return in_maps


def _calibrate_mu(feats, transitions, n=1000):
    E = np.exp(transitions.astype(np.float64))
    v = np.ones(NTAGS)
    acc = 0.0
    for t in range(n):
        v = (E.T * np.exp(feats[t].astype(np.float64))) @ v
        m = v.max()
        v /= m
        acc += np.log(m)
    return acc / n


def _extract_chunks(res_last):
    u = np.asarray(res_last, np.float64).reshape(CBLK, NTE, W, GW, NTE)
    return u.transpose(0, 2, 3, 1, 4).reshape(NB, NTE, NTE)[:NFULL]


def _combine_host(mats_per_core, feats, transitions, mu):
    """mats_per_core: list of 8 arrays [NFULL,3,3] (device full chunks).
    The per-core partial chunk (PREM steps) is computed here in f64; the
    global last chunk becomes the row-START boundary vector."""
    T = feats.shape[0]
    tr = np.asarray(transitions, np.float64)
    E3 = np.exp(tr[:NTE, :NTE] - mu)

    def host_chunk(off, ln):
        fb = np.exp(feats[off : off + ln, :NTE].astype(np.float64))
        U = np.eye(NTE)
        for t in range(ln):
            U = E3.T @ (fb[t][:, None] * U)
        return U

    per_core = []
    for k in range(8):
        # device ships rhs_{L-1}; apply the final E^T here (exact, f64)
        per_core.append(np.einsum("ki,bkj->bij", E3, mats_per_core[k]))
        off = k * T_CORE + NFULL * L
        if k < 7:
            per_core.append(host_chunk(off, PREM)[None])
    mats = np.concatenate(per_core, 0)

    # global last chunk: z^T = rowSTART(A_{T-1}) * prod(A_t desc) over PREM steps
    off_last = T - PREM
    e3 = np.exp(feats[off_last:, :NTE].astype(np.float64))
    z = np.exp(tr[:NTE, START] - mu) * e3[-1]
    for t in range(PREM - 2, -1, -1):
        z = e3[t] * (E3 @ z)
    Z = np.zeros((NTE, NTE))
    Z[0] = z

    Pm = np.concatenate([mats, Z[None]], axis=0)
    bad = ~np.isfinite(Pm).all(axis=(1, 2)) | (np.abs(Pm).max(axis=(1, 2)) <= 0)
    if bad.any():
        per_core_n = NFULL + 1
        for m in np.nonzero(bad)[0]:
            if m == len(Pm) - 1:
                raise RuntimeError("host z chunk non-finite")
            k, local = divmod(int(m), per_core_n)
            off = k * T_CORE + L * local
            ln = L if local < NFULL else PREM
            Pm[m] = host_chunk(off, ln)
    ls = 0.0
    n = np.abs(Pm).max(axis=(1, 2))
    Pm = Pm / n[:, None, None]
    ls += np.log(n).sum()
    while Pm.shape[0] > 1:
        nn_ = Pm.shape[0]
        m = nn_ - (nn_ % 2)
        C = np.einsum("bik,bkj->bij", Pm[1:m:2], Pm[0:m:2])
        if nn_ % 2:
            C = np.concatenate([C, Pm[m:]], 0)
        sc = np.abs(C).max(axis=(1, 2))
        sc = np.where(sc > 0, sc, 1.0)
        C /= sc[:, None, None]
        ls += np.log(sc).sum()
        Pm = C
    B = Pm[0]
    wt = np.exp(tr[STOP, :NTE])
    val = float(B[0] @ wt)
    return float(np.log(val) + ls + T * mu)


def _gold_score(feats, tags, transitions):
    T = feats.shape[0]
    tags_i = np.asarray(tags).astype(np.int64)
    tr = np.asarray(transitions, np.float64)
    prev = np.concatenate([np.array([START], np.int64), tags_i[:-1]])
    trans_score = tr[tags_i, prev].sum()
    emit_score = np.asarray(feats, np.float64)[np.arange(T), tags_i].sum()
    return trans_score + emit_score + tr[STOP, tags_i[-1]]


_PROGRAM = None


def _alpha_device(feats, transitions):
    global _PROGRAM
    from concourse.bass_utils import run_bass_kernel_spmd

    if _PROGRAM is None:
        _PROGRAM = _build_program()
    mu = _calibrate_mu(feats, transitions)
    in_maps = _prep_inputs(feats, transitions, mu)
    res = run_bass_kernel_spmd(_PROGRAM, in_maps, list(range(8)))
    mats = [_extract_chunks(res.results[k]["out_last"]) for k in range(8)]
    return _combine_host(mats, feats, transitions, mu)


def _alpha_numpy(feats, transitions):
    """Fallback: pairwise tree reduction over 5x5 operators (pure numpy)."""
    trans = transitions.astype(np.float32)
    M = trans[None, :, :] + feats[:, :, None].astype(np.float32)
    s = M.max(axis=(1, 2))
    Pr = np.exp(M - s[:, None, None])
    ls = s.astype(np.float64).sum()
    while Pr.shape[0] > 1:
        n = Pr.shape[0]
        m = n - (n % 2)
        C = (Pr[1:m:2][:, :, :, None] * Pr[0:m:2][:, None, :, :]).sum(axis=2)
        if n % 2:
            C = np.concatenate([C, Pr[m:]], axis=0)
        sc = C.max(axis=(1, 2))
        sc = np.where(sc > 0, sc, 1.0).astype(np.float32)
        C /= sc[:, None, None]
        ls += np.log(sc.astype(np.float64)).sum()
        Pr = C
    u = Pr[0][:, START].astype(np.float64)
    w = np.exp(trans[STOP].astype(np.float64))
    return float(np.log((w * u).sum()) + ls)


def kernel(feats, tags, transitions):
    feats = np.asarray(feats, np.float32)
    tags = np.asarray(tags)
    transitions = np.asarray(transitions, np.float32)

    alpha = None
    if feats.shape[0] == 2_000_000:
        for attempt in range(2):
            try:
                alpha = _alpha_device(feats, transitions)
                break
            except Exception:
                alpha = None
                if attempt == 0:
                    time.sleep(20)  # transient device wedges clear on retry
    if alpha is None:
        alpha = _alpha_numpy(feats, transitions)

    gold = _gold_score(feats, tags, transitions)
    return np.asarray(alpha - gold, dtype=np.float32)

